# revision 3
# baseline (speedup 1.0000x reference)
import sys
if "/opt/trn_rl_repo" not in sys.path:
    sys.path.insert(0, "/opt/trn_rl_repo")
import numpy as np

N_CORES = 8
B, T, H, E, L = 4, 2048, 3072, 4, 2
NTOK_TOTAL = B * T
NTOK = NTOK_TOTAL // N_CORES

_nc_cache = {}


def _get_nc():
    if "nc" not in _nc_cache:
        from moe_builder import build_moe_kernel
        _nc_cache["nc"] = build_moe_kernel(NTOK, H, E, L, 512)
    return _nc_cache["nc"]


def _round_fp22(a):
    """Round f32 to fp22 (13 explicit mantissa bits, RNE-ish) so the on-chip
    f32r conversion of We is an exact identity regardless of HW rounding mode."""
    u = np.ascontiguousarray(a, np.float32).view(np.uint32)
    return ((u + np.uint32(0x200)) & np.uint32(0xFFFFFC00)).view(np.float32)


def _surrogate_consts(ln_g, ln_b, Wr, br, We, be):
    """Host fp64 precompute for the exact layer-2 router surrogate:
    logits2 = rsig2*(x1@A - mu(x1)*sumA) + b2@Wr2 + br2 with
    x1@A = x@A + sum_e w_e (z@(We@A) + be@A)."""
    g1 = ln_g[0].astype(np.float64); b1 = ln_b[0].astype(np.float64)
    g2 = ln_g[1].astype(np.float64); b2 = ln_b[1].astype(np.float64)
    A = g2[:, None] * Wr[1].astype(np.float64)          # [H, E]
    A1 = A / g1[:, None]
    cols = [A1]
    for e in range(E):
        cols.append(We[0, e].astype(np.float64) @ A)    # [H, E]
    for e in range(E):
        cols.append(We[0, e].astype(np.float64).mean(axis=1)[:, None])
    Ucomb = np.concatenate(cols, axis=1).astype(np.float32)  # [H, 4+4E+E]
    rconst = np.zeros((8, E), np.float64)
    rconst[0] = b1 @ A1
    rconst[1] = A.sum(0)
    for e in range(E):
        rconst[2 + e] = be[0, e].astype(np.float64) @ A
    rconst[6] = [be[0, e].mean(dtype=np.float64) for e in range(E)]
    rconst[7] = b2 @ Wr[1].astype(np.float64) + br[1]
    return Ucomb, rconst.astype(np.float32)


def _make_in_maps(x, ln_g, ln_b, Wr, br, We, be):
    xf = np.ascontiguousarray(x.reshape(NTOK_TOTAL, H), dtype=np.float32)
    Ucomb, rconst = _surrogate_consts(ln_g, ln_b, Wr, br, We, be)
    shared = {
        "ln_g": np.ascontiguousarray(ln_g, np.float32),
        "ln_b": np.ascontiguousarray(ln_b, np.float32),
        "Wr": np.ascontiguousarray(Wr, np.float32),
        "br": np.ascontiguousarray(br, np.float32),
        "We": _round_fp22(We),
        "be": np.ascontiguousarray(be, np.float32),
        "Ucomb": Ucomb,
        "rconst": rconst,
    }
    return [
        {"x": xf[c * NTOK:(c + 1) * NTOK], **shared}
        for c in range(N_CORES)
    ]


def kernel(x, ln_g, ln_b, Wr, br, We, be):
    from concourse.bass_utils import run_bass_kernel_spmd
    nc = _get_nc()
    in_maps = _make_in_maps(x, ln_g, ln_b, Wr, br, We, be)
    res = run_bass_kernel_spmd(nc, in_maps, core_ids=list(range(N_CORES)))
    y = np.concatenate([res.results[c]["y"] for c in range(N_CORES)], axis=0)
    return y.reshape(B, T, H).astype(np.float32)


def run_profiled(inputs):
    from concourse.bass_utils import run_bass_kernel_spmd
    nc = _get_nc()
    in_maps = _make_in_maps(**inputs)
    return run_bass_kernel_spmd(nc, in_maps, core_ids=list(range(N_CORES)),
                                trace=True)
